# revision 17
# baseline (speedup 1.0000x reference)
"""GAT layer kernel for Trainium2, sharded across 8 NeuronCores.

Math: reference computes
    h = x @ W.T;  e_ij = (h @ a1)[i] + (h @ a2)[j];  mask by adj;
    softmax over j; out = attn @ h.
Because e_i is constant along the softmax axis it cancels, so with
w_j = exp(h_j . a2):
    out[i] = sum_j adj[i,j] * w_j * h[j] / sum_j adj[i,j] * w_j
a1 is mathematically irrelevant.

Design (v15 = v14 + fp8 phase-1):
  * adjacency is packed host-side to fp8e4 BIT PATTERNS (0x00 / 0x38 =
    1.0), pre-arranged per 512-row super-tile into the exact [p, b, i,
    r] device layout so every adj DMA is a pure contiguous copy.
  * THREE bulk DMA streams on separate engine queues, because half A
    consumes adj at ~240 GB/s + x at ~120 GB/s, more than one queue
    delivers: x on the gpsimd SWDGE ring (512KB pieces, first chunk
    split finer for priming), even adj super-tiles on the (otherwise
    idle) sync HWDGE ring emitted upfront, odd adj super-tiles on the
    scalar HWDGE ring with emissions interleaved into the loop so
    they never delay the scalar chain ops.
  * phase 1: h quarters (2 j-chunks) via bf16 matmuls into a PSUM
    scratch rotating over two 2-bank buffers; chain = batched Exp
    (scalar), w-column fp8 cast (scalar), single DVE broadcast
    multiply -> fp8 [w*h | w] tile. Phase-2 consumption lags THREE
    super-chunks so the chain never stalls the PE.
    ws = exp(e + ln(1/8)); the 1/8 keeps w*h inside fp8e4 range and
    cancels in num/den.
  * phase 2 TRANSPOSED vs v10: stationary = adj^T chunk [j:256,
    r:128] (fp8 DoubleRow k=256), moving = [w*h | w] [j:256, 257
    cols]. out[r, 0:256] = numerator rows, out[r, 256] = the
    denominator - it rides along as moving column 257, so v10's
    separate denominator matmuls (a third of the phase-2 moving
    cycles) vanish.
  * PSUM: 4 rotating scratch banks + 4 rotating accumulator banks.
    Rows go in two halves of 4 row-chunks: half A (rows 0:512) fused
    with phase 1; half B (rows 512:1024) a pure phase-2 sweep over
    the SBUF-resident adj tiles, reusing half A's banks right after
    the epilogue-A reads.
  * epilogue per half: DVE reciprocal of the [128, 1] denominator
    column (num and den share the partition = output row), then
    per-partition scale multiplies into one [128, 4, 256] fp16 tile,
    shipped by a single DMA into the final [1024, 256] layout. No
    host math beyond a concat/cast.

Measured numerics: rel err ~9.7e-3 vs fp32 reference (tolerance 2e-2).
"""

import sys

import numpy as np

for _p in ("/opt/trn_rl_repo",):
    try:
        import concourse.bass  # noqa: F401

        break
    except ImportError:
        if _p not in sys.path:
            sys.path.insert(0, _p)

import ml_dtypes

import concourse.bass as bass
import concourse.mybir as mybir
import concourse.tile as tile
from concourse.bass_utils import run_bass_kernel_spmd

dt = mybir.dt
AF = mybir.ActivationFunctionType
PM = mybir.MatmulPerfMode

N = 8192
D = 256
NCORES = 8
RB = N // NCORES  # 1024 output rows per core
W_FREE = 260  # 256 h cols + 1 e col + 3 pad
NJ = N // 128  # 64 j-chunks
NJS = N // 256  # 32 j-super-chunks (DoubleRow k=256)
NJP = NJS // 2  # 16 adj DMA transfers (2 super-chunks each)
HW_FREE = 260  # 256 w*h cols + 1 w col + 3 pad
LAG = 5  # phase-2 consumption lag (in super-chunks) behind phase 1
LOG_S = float(np.log(1.0 / 128.0))  # w scale: num/den both end up /8
ESC = 1.0 / 16.0  # Exp input scale: undoes the x16 on W (fp8 normal range)

# ---------------------------------------------------------------------------
# walrus in this container accepts at most ONE sync-wait command on several
# instruction structs (Drain, 4-byte self-loading Matmult, ...) while the
# newer Tile scheduler emits more. Split the extras into single-wait
# EventSemaphore prefixes on the same engine (identical semantics).
_ev_counter = [0]


def _legalize_multiwait(nc, max_keep=1):
    for f in nc.m.functions:
        for bb in f.blocks:
            il = bb.instructions
            idx = 0
            while idx < len(il):
                inst = il[idx]
                si = inst.sync_info
                if si is not None and si.on_wait and len(si.on_wait) > max_keep:
                    waits = list(si.on_wait)
                    keep = waits[len(waits) - max_keep :] if max_keep else []
                    extra = waits[: len(waits) - max_keep] if max_keep else waits
                    si.on_wait = keep
                    for w in extra:
                        _ev_counter[0] += 1
                        ev = mybir.InstEventSemaphore(
                            name=f"lgw_{_ev_counter[0]}", ins=[], outs=[]
                        )
                        ev.engine = inst.engine
                        ev.sync_info = mybir.SyncInfo(on_wait=[w], on_update=[])
                        il.insert(idx, ev)
                        idx += 1
                idx += 1


# ---------------------------------------------------------------------------


def _build_program():
    nc = bass.Bass("TRN2", debug=False)

    # x packed host-side to fp8 in DoubleRow form: xP8[p, ic, j] = x[j, ic*128+p]
    xP8 = nc.dram_tensor("xP8", [128, 2, N], dt.float8e4, kind="ExternalInput").ap()
    # [16*W.T | 16*W.T@a2] in the same [p, ic, col] DoubleRow form, plus
    # the fp8 quantization RESIDUAL of that matrix: a second accumulating
    # matmul with it cuts the coherent (non-averaging) W quantization error
    # from ~3.5% to ~0.1%.
    WTe8 = nc.dram_tensor(
        "WTe8", [128, 2, W_FREE], dt.float8e4, kind="ExternalInput"
    ).ap()
    WTe8r = nc.dram_tensor(
        "WTe8r", [128, 2, W_FREE], dt.float8e4, kind="ExternalInput"
    ).ap()
    # adj rows of this core, packed host-side into the device tile layout:
    # adjP8[jp, p, b, i, r] = adj^T fp8 pattern for j = jp*512 + b*256 +
    # i*128 + p; each [p, b, i, r] tile is one contiguous 512KB DMA.
    adjP8 = nc.dram_tensor(
        "adjP8", [NJP, 128, 2, 2, RB], dt.float8e4, kind="ExternalInput"
    ).ap()
    # final output rows of this core (fp16 is plenty: |out| <= max|h|,
    # 5e-4 rel step vs 1e-2 budget)
    outO = nc.dram_tensor("outO", [RB, D], dt.float16, kind="ExternalOutput").ap()

    XCH = 1024  # x streamed in [128, 2, XCH] fp8 chunks (one DMA each)
    NXB = N // XCH  # 4 chunks per i-half
    NCPB = XCH // 128  # 16 j-chunks per x chunk

    with tile.TileContext(nc) as tc:
        with (
            tc.tile_pool(name="xr", bufs=1) as xr_pool,
            tc.tile_pool(name="wte", bufs=1) as wte_pool,
            tc.tile_pool(name="hw8", bufs=1) as hw8_pool,
            tc.tile_pool(name="wcol", bufs=4) as w_pool,
            tc.tile_pool(name="adjr", bufs=16) as adj_pool,
            tc.tile_pool(name="eps", bufs=4) as ep_pool,
        ):
            wte8 = wte_pool.tile([128, 2, W_FREE], dt.float8e4, name="wte8")
            nc.scalar.dma_start(wte8, WTe8)
            wte8r = wte_pool.tile([128, 2, W_FREE], dt.float8e4, name="wte8r")
            nc.scalar.dma_start(wte8r, WTe8r)

            xr = [None] * NXB
            at_tiles = [None] * NJP

            # one whole x chunk per DMA: readers wait on every writer of a
            # tile (tile-granular tracking), so finer chunks - not split
            # DMAs - are what actually shortens the critical path.
            def load_x(b, eng):
                eng.dma_start(xr[b], xP8[:, :, b * XCH : (b + 1) * XCH])

            def load_at(jp, engs):
                # one engine: single 512KB DMA; two engines: the two halves
                # land in parallel on both queues (consumers wait for the
                # whole tile either way).
                at = adj_pool.tile(
                    [128, 2, 2, RB], dt.float8e4, name=f"at{jp}", tag="at"
                )
                src = adjP8[jp].rearrange("p b i r -> p (b i r)")
                dst = at.rearrange("p b i r -> p (b i r)")
                step = (2 * 2 * RB) // len(engs)
                for s, eng in enumerate(engs):
                    eng.dma_start(
                        dst[:, s * step : (s + 1) * step],
                        src[:, s * step : (s + 1) * step],
                    )
                at_tiles[jp] = at

            for b in range(NXB):
                xr[b] = xr_pool.tile(
                    [128, 2, XCH], dt.float8e4, name=f"xr{b}", tag="x", bufs=4
                )

            # Bulk DMA split across two rings, each in earliest-deadline
            # order. The serialized sync HWDGE ring (~205 GB/s effective,
            # otherwise idle) carries every other adj super-tile upfront;
            # the gpsimd SWDGE ring (~270 GB/s) interleaves the x pieces
            # with the remaining adj tiles.
            load_x(0, nc.sync)
            load_x(1, nc.sync)
            load_at(0, (nc.sync, nc.gpsimd))
            load_x(2, nc.gpsimd)
            load_at(1, (nc.gpsimd,))
            load_at(2, (nc.sync,))
            load_x(3, nc.gpsimd)
            load_at(3, (nc.gpsimd,))
            load_at(4, (nc.sync,))
            load_x(4, nc.gpsimd)
            load_at(5, (nc.gpsimd,))
            load_at(6, (nc.sync,))
            load_x(5, nc.gpsimd)
            load_at(7, (nc.gpsimd,))
            load_at(8, (nc.sync,))
            load_x(6, nc.gpsimd)
            load_at(9, (nc.gpsimd,))
            load_at(10, (nc.sync,))
            load_x(7, nc.gpsimd)
            load_at(11, (nc.gpsimd,))
            load_at(12, (nc.sync,))
            load_at(13, (nc.gpsimd,))
            load_at(14, (nc.sync,))
            load_at(15, (nc.sync,))

            # fp8 moving tile for phase 2:
            # hw8_all[:, jc, d] = (w*h/8)[j = jc*128 + p, d]  for d < 256
            # hw8_all[:, jc, 256] = (w/8)[j]   (the denominator column)
            hw8_all = hw8_pool.tile([128, NJ, HW_FREE], dt.float8e4, name="hw8_all")
            w_all = w_pool.tile([128, NJ], dt.float32, name="w_all")
            bias_s = w_pool.tile([128, 1], dt.float32, name="bias_s")
            nc.vector.memset(bias_s, LOG_S)

            # ---- fused loop: PSUM = 2x2 rotating scratch banks + 4
            # rotating accumulator banks = exactly 8.
            with (
                tc.tile_pool(name="ph", bufs=1, space="PSUM") as ph_pool,
                tc.tile_pool(name="acc", bufs=1, space="PSUM") as acc_pool,
            ):
                ph_bufs = [
                    ph_pool.tile([128, 2, 512], dt.float32, name=f"ph_q{i}",
                                 tag="ph", bufs=2)
                    for i in range(2)
                ]
                # HAM warm-up: dependency-free matmuls on uninitialized SBUF
                # while the first x/adj DMAs are in flight, so the PE clock
                # gate is already at 8/8 when real work starts. Garbage
                # results land in ph buf 0 and are overwritten by the
                # first real start=True matmul.
                warm = hw8_pool.tile([128, 640], dt.bfloat16, name="warm")
                nc.vector.memset(warm, 0.0)
                for _ in range(3):
                    nc.tensor.matmul(
                        ph_bufs[0][:, 0, :],
                        warm[:, 0:128],
                        warm[:, 128:640],
                        start=True,
                        stop=True,
                    )

                def emit_quarter(q):
                    # h/e matmuls for j-chunks 2q, 2q+1 into the rotating
                    # scratch, then ws = exp(e)/8 and the fp8 moving tile.
                    ph_q = ph_bufs[q % 2]
                    for k in range(2):
                        jc = 2 * q + k
                        b, sl = jc // NCPB, bass.ts(jc % NCPB, 128)
                        for wt, st in ((wte8, True), (wte8r, False)):
                            nc.tensor.matmul(
                                ph_q[:, k, 0:W_FREE],
                                xr[b][:, :, sl],
                                wt,
                                start=st,
                                stop=not st,
                                perf_mode=PM.DoubleRow,
                            )
                    j0 = 2 * q
                    nc.scalar.activation(
                        w_all[:, j0 : j0 + 2],
                        ph_q[:, :, 256],
                        AF.Exp,
                        bias=bias_s[:, 0:1],
                        scale=ESC,
                    )
                    # den column = 16*w_all = w/8  (ph holds 16h, w_all = w/128,
                    # so hw8 = ph*w_all = w*h/8 and num/den matches exactly)
                    nc.scalar.activation(
                        hw8_all[:, j0 : j0 + 2, 256],
                        w_all[:, j0 : j0 + 2],
                        AF.Copy,
                        scale=16.0,
                    )
                    nc.vector.tensor_tensor(
                        hw8_all[:, j0 : j0 + 2, 0:256],
                        ph_q[:, :, 0:256],
                        w_all[:, j0 : j0 + 2].to_broadcast([128, 2, 256]),
                        mybir.AluOpType.mult,
                    )

                def emit_js(js, accs, rc0):
                    # phase-2 matmuls for super-chunk js, row-chunks
                    # rc0..rc0+3: stationary = adj^T [256, 128], moving =
                    # [w*h | w] [256, 257]; out[r, 0:256]=num, out[r,256]=den
                    at = at_tiles[js // 2][:, js % 2]  # [128, 2, RB]
                    st, sp = js == 0, js == NJS - 1
                    mov = hw8_all[:, 2 * js : 2 * js + 2, 0:257]
                    for k, acc in enumerate(accs):
                        rc = rc0 + k
                        nc.tensor.matmul(
                            acc[:, 0:257],
                            at[:, :, rc * 128 : (rc + 1) * 128],
                            mov,
                            start=st,
                            stop=sp,
                            perf_mode=PM.DoubleRow,
                            skip_group_check=True,
                        )

                def emit_epilogue(accs, rc0, eng, eng2=None):
                    # num and den share the partition (= output row): DVE
                    # reciprocal of the [128,1] den column, then per-
                    # partition scale multiplies into one fp16 tile; the
                    # first pair ships while the second is still computing.
                    ob = ep_pool.tile([128, 4, 256], dt.float16, name="ob", tag="ob")
                    for k, acc in enumerate(accs):
                        rec = ep_pool.tile([128, 1], dt.float32, name="rec", tag="rec")
                        nc.vector.reciprocal(rec, acc[:, 256:257])
                        if k % 2 == 0:
                            nc.scalar.activation(
                                ob[:, k, :], acc[:, 0:256], AF.Copy, scale=rec[:, 0:1]
                            )
                        else:
                            nc.vector.tensor_scalar_mul(
                                ob[:, k, :], acc[:, 0:256], rec[:, 0:1]
                            )
                        if k == 1:
                            dst = outO[rc0 * 128 : (rc0 + 2) * 128, :].rearrange(
                                "(c p) d -> p c d", p=128
                            )
                            eng.dma_start(dst, ob[:, 0:2])
                    dst = outO[(rc0 + 2) * 128 : (rc0 + 4) * 128, :].rearrange(
                        "(c p) d -> p c d", p=128
                    )
                    (eng2 or eng).dma_start(dst, ob[:, 2:4])

                # ---- half A: rows 0:512 fused with phase 1; phase-2
                # consumption lags LAG super-chunks so the convert chain
                # never stalls the PE. Odd adj tiles are emitted on the
                # scalar ring two tiles ahead of consumption.
                accA = [
                    acc_pool.tile([128, 512], dt.float32, name=f"accA{rc}",
                                  tag="acc", bufs=4)
                    for rc in range(4)
                ]
                for q in range(NJS + LAG):
                    if q < NJS:
                        if q >= LAG:
                            emit_js(q - LAG, accA, 0)
                        emit_quarter(q)
                    else:
                        emit_js(q - LAG, accA, 0)
                emit_epilogue(accA, 0, nc.sync)

                # ---- half B: rows 512:1024, pure phase-2 sweep; the
                # accumulators rotate onto half A's banks, whose epilogue
                # reads are already done (they were emitted first).
                accB = [
                    acc_pool.tile([128, 512], dt.float32, name=f"accB{rc}",
                                  tag="acc", bufs=4)
                    for rc in range(4)
                ]
                for js in range(NJS):
                    emit_js(js, accB, 4)
                emit_epilogue(accB, 4, nc.sync, nc.scalar)

    _legalize_multiwait(nc, max_keep=1)
    return nc


_CACHED = {}


def _prep_inputs(x, adj, W, a):
    # x and 16*W packed to fp8 in DoubleRow form [p, ic, .] (d = ic*128+p);
    # the x16 keeps W's small entries out of the fp8 subnormal range and is
    # undone by the Exp input scale / folded into the w column scale.
    xP8 = np.ascontiguousarray(
        x.T.reshape(2, 128, N).transpose(1, 0, 2)
    ).astype(ml_dtypes.float8_e4m3)
    WTe = np.zeros((D, W_FREE), dtype=np.float32)
    WTe[:, :256] = W.T * 16.0
    WTe[:, 256] = (W.T.astype(np.float64) @ a[256:].astype(np.float64)).astype(
        np.float32
    ) * 16.0
    WTeP = np.ascontiguousarray(WTe.reshape(2, 128, W_FREE).transpose(1, 0, 2))
    WTe8 = WTeP.astype(ml_dtypes.float8_e4m3)
    WTe8r = (WTeP - WTe8.astype(np.float32)).astype(ml_dtypes.float8_e4m3)
    # adjacency -> fp8e4 bit patterns (0x00 / 0x38 == 1.0), pre-arranged
    # per core into the [jp, p, b, i, r] device tile layout (j = jp*512 +
    # b*256 + i*128 + p) so every adj DMA is a contiguous copy.
    adj8 = np.where(adj != 0, np.uint8(0x38), np.uint8(0))
    in_maps = []
    for c in range(NCORES):
        adjT_c = np.ascontiguousarray(adj8[c * RB : (c + 1) * RB, :].T)  # [N, RB]
        adjP = adjT_c.reshape(NJP, 2, 2, 128, RB).transpose(0, 3, 1, 2, 4)
        adjP = np.ascontiguousarray(adjP).view(ml_dtypes.float8_e4m3)
        in_maps.append(
            {"xP8": xP8, "WTe8": WTe8, "WTe8r": WTe8r, "adjP8": adjP}
        )
    return in_maps


def _run(in_maps, **kw):
    if "nc" not in _CACHED:
        _CACHED["nc"] = _build_program()
    # The device occasionally comes up wedged (NRT_EXEC_UNIT_UNRECOVERABLE)
    # from a previous process; one retry after a short pause recovers it.
    import time as _time

    last_err = None
    for attempt in range(3):
        try:
            return run_bass_kernel_spmd(
                _CACHED["nc"], in_maps, core_ids=list(range(NCORES)), **kw
            )
        except Exception as e:  # noqa: BLE001
            last_err = e
            if "UNRECOVERABLE" not in str(e) and "UNAVAILABLE" not in str(e):
                raise
            _time.sleep(3.0)
    raise last_err


def _assemble(results):
    blocks = [np.asarray(r["outO"], dtype=np.float32) for r in results]
    return np.concatenate(blocks, axis=0)


def kernel(x, adj, W, a):
    in_maps = _prep_inputs(x, adj, W, a)
    res = _run(in_maps)
    return _assemble(res.results)


# revision 18
# speedup vs baseline: 1.1670x; 1.1670x over previous
"""GAT layer kernel for Trainium2, sharded across 8 NeuronCores.

Math: reference computes
    h = x @ W.T;  e_ij = (h @ a1)[i] + (h @ a2)[j];  mask by adj;
    softmax over j; out = attn @ h.
Because e_i is constant along the softmax axis it cancels, so with
w_j = exp(h_j . a2):
    out[i] = sum_j adj[i,j] * w_j * h[j] / sum_j adj[i,j] * w_j
a1 is mathematically irrelevant.

Design (v15 = v14 + fp8 phase-1):
  * adjacency is packed host-side to fp8e4 BIT PATTERNS (0x00 / 0x38 =
    1.0), pre-arranged per 512-row super-tile into the exact [p, b, i,
    r] device layout so every adj DMA is a pure contiguous copy.
  * THREE bulk DMA streams on separate engine queues, because half A
    consumes adj at ~240 GB/s + x at ~120 GB/s, more than one queue
    delivers: x on the gpsimd SWDGE ring (512KB pieces, first chunk
    split finer for priming), even adj super-tiles on the (otherwise
    idle) sync HWDGE ring emitted upfront, odd adj super-tiles on the
    scalar HWDGE ring with emissions interleaved into the loop so
    they never delay the scalar chain ops.
  * phase 1: h quarters (2 j-chunks) via bf16 matmuls into a PSUM
    scratch rotating over two 2-bank buffers; chain = batched Exp
    (scalar), w-column fp8 cast (scalar), single DVE broadcast
    multiply -> fp8 [w*h | w] tile. Phase-2 consumption lags THREE
    super-chunks so the chain never stalls the PE.
    ws = exp(e + ln(1/8)); the 1/8 keeps w*h inside fp8e4 range and
    cancels in num/den.
  * phase 2 TRANSPOSED vs v10: stationary = adj^T chunk [j:256,
    r:128] (fp8 DoubleRow k=256), moving = [w*h | w] [j:256, 257
    cols]. out[r, 0:256] = numerator rows, out[r, 256] = the
    denominator - it rides along as moving column 257, so v10's
    separate denominator matmuls (a third of the phase-2 moving
    cycles) vanish.
  * PSUM: 4 rotating scratch banks + 4 rotating accumulator banks.
    Rows go in two halves of 4 row-chunks: half A (rows 0:512) fused
    with phase 1; half B (rows 512:1024) a pure phase-2 sweep over
    the SBUF-resident adj tiles, reusing half A's banks right after
    the epilogue-A reads.
  * epilogue per half: DVE reciprocal of the [128, 1] denominator
    column (num and den share the partition = output row), then
    per-partition scale multiplies into one [128, 4, 256] fp16 tile,
    shipped by a single DMA into the final [1024, 256] layout. No
    host math beyond a concat/cast.

Measured numerics: rel err ~9.7e-3 vs fp32 reference (tolerance 2e-2).
"""

import sys

import numpy as np

for _p in ("/opt/trn_rl_repo",):
    try:
        import concourse.bass  # noqa: F401

        break
    except ImportError:
        if _p not in sys.path:
            sys.path.insert(0, _p)

import ml_dtypes

import concourse.bass as bass
import concourse.mybir as mybir
import concourse.tile as tile
from concourse.bass_utils import run_bass_kernel_spmd

dt = mybir.dt
AF = mybir.ActivationFunctionType
PM = mybir.MatmulPerfMode

N = 8192
D = 256
NCORES = 8
RB = N // NCORES  # 1024 output rows per core
W_FREE = 260  # 256 h cols + 1 e col + 3 pad
NJ = N // 128  # 64 j-chunks
NJS = N // 256  # 32 j-super-chunks (DoubleRow k=256)
NJP = NJS // 2  # 16 adj DMA transfers (2 super-chunks each)
HW_FREE = 260  # 256 w*h cols + 1 w col + 3 pad
LAG = 5  # phase-2 consumption lag (in super-chunks) behind phase 1
LOG_S = float(np.log(1.0 / 128.0))  # w scale: num/den both end up /8
ESC = 1.0 / 16.0  # Exp input scale: undoes the x16 on W (fp8 normal range)

# ---------------------------------------------------------------------------
# walrus in this container accepts at most ONE sync-wait command on several
# instruction structs (Drain, 4-byte self-loading Matmult, ...) while the
# newer Tile scheduler emits more. Split the extras into single-wait
# EventSemaphore prefixes on the same engine (identical semantics).
_ev_counter = [0]


def _legalize_multiwait(nc, max_keep=1):
    for f in nc.m.functions:
        for bb in f.blocks:
            il = bb.instructions
            idx = 0
            while idx < len(il):
                inst = il[idx]
                si = inst.sync_info
                if si is not None and si.on_wait and len(si.on_wait) > max_keep:
                    waits = list(si.on_wait)
                    keep = waits[len(waits) - max_keep :] if max_keep else []
                    extra = waits[: len(waits) - max_keep] if max_keep else waits
                    si.on_wait = keep
                    for w in extra:
                        _ev_counter[0] += 1
                        ev = mybir.InstEventSemaphore(
                            name=f"lgw_{_ev_counter[0]}", ins=[], outs=[]
                        )
                        ev.engine = inst.engine
                        ev.sync_info = mybir.SyncInfo(on_wait=[w], on_update=[])
                        il.insert(idx, ev)
                        idx += 1
                idx += 1


# ---------------------------------------------------------------------------


def _build_program():
    nc = bass.Bass("TRN2", debug=False)

    # x packed host-side to fp8 in DoubleRow form: xP8[p, ic, j] = x[j, ic*128+p]
    xP8 = nc.dram_tensor("xP8", [128, 2, N], dt.float8e4, kind="ExternalInput").ap()
    # [16*W.T | 16*W.T@a2] in the same [p, ic, col] DoubleRow form, plus
    # the fp8 quantization RESIDUAL of that matrix: a second accumulating
    # matmul with it cuts the coherent (non-averaging) W quantization error
    # from ~3.5% to ~0.1%.
    WTe8 = nc.dram_tensor(
        "WTe8", [128, 2, W_FREE], dt.float8e4, kind="ExternalInput"
    ).ap()
    WTe8r = nc.dram_tensor(
        "WTe8r", [128, 2, W_FREE], dt.float8e4, kind="ExternalInput"
    ).ap()
    # adj rows of this core, packed host-side into the device tile layout:
    # adjP8[jp, p, b, i, r] = adj^T fp8 pattern for j = jp*512 + b*256 +
    # i*128 + p; each [p, b, i, r] tile is one contiguous 512KB DMA.
    adjP8 = nc.dram_tensor(
        "adjP8", [NJP, 128, 2, 2, RB], dt.float8e4, kind="ExternalInput"
    ).ap()
    # final output rows of this core (fp16 is plenty: |out| <= max|h|,
    # 5e-4 rel step vs 1e-2 budget)
    outO = nc.dram_tensor("outO", [RB, D], dt.float16, kind="ExternalOutput").ap()

    XCH = 1024  # x streamed in [128, 2, XCH] fp8 chunks (one DMA each)
    NXB = N // XCH  # 4 chunks per i-half
    NCPB = XCH // 128  # 16 j-chunks per x chunk

    with tile.TileContext(nc) as tc:
        with (
            tc.tile_pool(name="xr", bufs=1) as xr_pool,
            tc.tile_pool(name="wte", bufs=1) as wte_pool,
            tc.tile_pool(name="hw8", bufs=1) as hw8_pool,
            tc.tile_pool(name="wcol", bufs=4) as w_pool,
            tc.tile_pool(name="adjr", bufs=16) as adj_pool,
            tc.tile_pool(name="eps", bufs=4) as ep_pool,
        ):
            wte8 = wte_pool.tile([128, 2, W_FREE], dt.float8e4, name="wte8")
            nc.scalar.dma_start(wte8, WTe8)
            wte8r = wte_pool.tile([128, 2, W_FREE], dt.float8e4, name="wte8r")
            nc.scalar.dma_start(wte8r, WTe8r)

            xr = [None] * NXB
            at_tiles = [None] * NJP

            # one whole x chunk per DMA: readers wait on every writer of a
            # tile (tile-granular tracking), so finer chunks - not split
            # DMAs - are what actually shortens the critical path.
            def load_x(b, eng):
                eng.dma_start(xr[b], xP8[:, :, b * XCH : (b + 1) * XCH])

            def load_at(jp, engs):
                # one engine: single 512KB DMA; two engines: the two halves
                # land in parallel on both queues (consumers wait for the
                # whole tile either way).
                at = adj_pool.tile(
                    [128, 2, 2, RB], dt.float8e4, name=f"at{jp}", tag="at"
                )
                src = adjP8[jp].rearrange("p b i r -> p (b i r)")
                dst = at.rearrange("p b i r -> p (b i r)")
                step = (2 * 2 * RB) // len(engs)
                for s, eng in enumerate(engs):
                    eng.dma_start(
                        dst[:, s * step : (s + 1) * step],
                        src[:, s * step : (s + 1) * step],
                    )
                at_tiles[jp] = at

            for b in range(NXB):
                xr[b] = xr_pool.tile(
                    [128, 2, XCH], dt.float8e4, name=f"xr{b}", tag="x", bufs=4
                )

            # Bulk DMA split across two rings, each in earliest-deadline
            # order. The serialized sync HWDGE ring (~205 GB/s effective,
            # otherwise idle) carries every other adj super-tile upfront;
            # the gpsimd SWDGE ring (~270 GB/s) interleaves the x pieces
            # with the remaining adj tiles.
            load_x(0, nc.sync)
            load_at(0, (nc.sync, nc.gpsimd))
            load_x(1, nc.gpsimd)
            load_at(1, (nc.sync,))
            load_at(2, (nc.gpsimd,))
            load_x(2, nc.gpsimd)
            load_at(3, (nc.sync,))
            load_x(3, nc.gpsimd)
            load_x(4, nc.sync)
            load_at(4, (nc.gpsimd,))
            load_at(5, (nc.sync,))
            load_x(5, nc.gpsimd)
            load_at(6, (nc.gpsimd,))
            load_at(7, (nc.sync,))
            load_x(6, nc.gpsimd)
            load_at(8, (nc.gpsimd,))
            load_at(9, (nc.sync,))
            load_x(7, nc.gpsimd)
            load_at(10, (nc.gpsimd,))
            load_at(11, (nc.sync,))
            load_at(12, (nc.gpsimd,))
            load_at(13, (nc.sync,))
            load_at(14, (nc.sync,))
            load_at(15, (nc.sync,))

            # fp8 moving tile for phase 2:
            # hw8_all[:, jc, d] = (w*h/8)[j = jc*128 + p, d]  for d < 256
            # hw8_all[:, jc, 256] = (w/8)[j]   (the denominator column)
            hw8_all = hw8_pool.tile([128, NJ, HW_FREE], dt.float8e4, name="hw8_all")
            w_all = w_pool.tile([128, NJ], dt.float32, name="w_all")
            bias_s = w_pool.tile([128, 1], dt.float32, name="bias_s")
            nc.vector.memset(bias_s, LOG_S)

            # ---- fused loop: PSUM = 2x2 rotating scratch banks + 4
            # rotating accumulator banks = exactly 8.
            with (
                tc.tile_pool(name="ph", bufs=1, space="PSUM") as ph_pool,
                tc.tile_pool(name="acc", bufs=1, space="PSUM") as acc_pool,
            ):
                ph_bufs = [
                    ph_pool.tile([128, 2, 512], dt.float32, name=f"ph_q{i}",
                                 tag="ph", bufs=2)
                    for i in range(2)
                ]
                # HAM warm-up: dependency-free matmuls on uninitialized SBUF
                # while the first x/adj DMAs are in flight, so the PE clock
                # gate is already at 8/8 when real work starts. Garbage
                # results land in ph buf 0 and are overwritten by the
                # first real start=True matmul.
                warm = hw8_pool.tile([128, 640], dt.bfloat16, name="warm")
                nc.vector.memset(warm, 0.0)
                for _ in range(3):
                    nc.tensor.matmul(
                        ph_bufs[0][:, 0, :],
                        warm[:, 0:128],
                        warm[:, 128:640],
                        start=True,
                        stop=True,
                    )

                def emit_quarter(q):
                    # h/e matmuls for j-chunks 2q, 2q+1 into the rotating
                    # scratch, then ws = exp(e)/8 and the fp8 moving tile.
                    ph_q = ph_bufs[q % 2]
                    for k in range(2):
                        jc = 2 * q + k
                        b, sl = jc // NCPB, bass.ts(jc % NCPB, 128)
                        for wt, st in ((wte8, True), (wte8r, False)):
                            nc.tensor.matmul(
                                ph_q[:, k, 0:W_FREE],
                                xr[b][:, :, sl],
                                wt,
                                start=st,
                                stop=not st,
                                perf_mode=PM.DoubleRow,
                            )
                    j0 = 2 * q
                    nc.scalar.activation(
                        w_all[:, j0 : j0 + 2],
                        ph_q[:, :, 256],
                        AF.Exp,
                        bias=bias_s[:, 0:1],
                        scale=ESC,
                    )
                    # den column = 16*w_all = w/8  (ph holds 16h, w_all = w/128,
                    # so hw8 = ph*w_all = w*h/8 and num/den matches exactly)
                    nc.scalar.activation(
                        hw8_all[:, j0 : j0 + 2, 256],
                        w_all[:, j0 : j0 + 2],
                        AF.Copy,
                        scale=16.0,
                    )
                    nc.vector.tensor_tensor(
                        hw8_all[:, j0 : j0 + 2, 0:256],
                        ph_q[:, :, 0:256],
                        w_all[:, j0 : j0 + 2].to_broadcast([128, 2, 256]),
                        mybir.AluOpType.mult,
                    )

                def emit_js(js, accs, rc0):
                    # phase-2 matmuls for super-chunk js, row-chunks
                    # rc0..rc0+3: stationary = adj^T [256, 128], moving =
                    # [w*h | w] [256, 257]; out[r, 0:256]=num, out[r,256]=den
                    at = at_tiles[js // 2][:, js % 2]  # [128, 2, RB]
                    st, sp = js == 0, js == NJS - 1
                    mov = hw8_all[:, 2 * js : 2 * js + 2, 0:257]
                    for k, acc in enumerate(accs):
                        rc = rc0 + k
                        nc.tensor.matmul(
                            acc[:, 0:257],
                            at[:, :, rc * 128 : (rc + 1) * 128],
                            mov,
                            start=st,
                            stop=sp,
                            perf_mode=PM.DoubleRow,
                            skip_group_check=True,
                        )

                def emit_epilogue(accs, rc0, eng, eng2=None):
                    # num and den share the partition (= output row): DVE
                    # reciprocal of the [128,1] den column, then per-
                    # partition scale multiplies into one fp16 tile; the
                    # first pair ships while the second is still computing.
                    ob = ep_pool.tile([128, 4, 256], dt.float16, name="ob", tag="ob")
                    for k, acc in enumerate(accs):
                        rec = ep_pool.tile([128, 1], dt.float32, name="rec", tag="rec")
                        nc.vector.reciprocal(rec, acc[:, 256:257])
                        if k % 2 == 0:
                            nc.scalar.activation(
                                ob[:, k, :], acc[:, 0:256], AF.Copy, scale=rec[:, 0:1]
                            )
                        else:
                            nc.vector.tensor_scalar_mul(
                                ob[:, k, :], acc[:, 0:256], rec[:, 0:1]
                            )
                        if k == 1:
                            dst = outO[rc0 * 128 : (rc0 + 2) * 128, :].rearrange(
                                "(c p) d -> p c d", p=128
                            )
                            eng.dma_start(dst, ob[:, 0:2])
                    dst = outO[(rc0 + 2) * 128 : (rc0 + 4) * 128, :].rearrange(
                        "(c p) d -> p c d", p=128
                    )
                    (eng2 or eng).dma_start(dst, ob[:, 2:4])

                # ---- half A: rows 0:512 fused with phase 1; phase-2
                # consumption lags LAG super-chunks so the convert chain
                # never stalls the PE. Odd adj tiles are emitted on the
                # scalar ring two tiles ahead of consumption.
                accA = [
                    acc_pool.tile([128, 512], dt.float32, name=f"accA{rc}",
                                  tag="acc", bufs=4)
                    for rc in range(4)
                ]
                for q in range(NJS + LAG):
                    if q < NJS:
                        if q >= LAG:
                            emit_js(q - LAG, accA, 0)
                        emit_quarter(q)
                    else:
                        emit_js(q - LAG, accA, 0)
                emit_epilogue(accA, 0, nc.sync)

                # ---- half B: rows 512:1024, pure phase-2 sweep; the
                # accumulators rotate onto half A's banks, whose epilogue
                # reads are already done (they were emitted first).
                accB = [
                    acc_pool.tile([128, 512], dt.float32, name=f"accB{rc}",
                                  tag="acc", bufs=4)
                    for rc in range(4)
                ]
                for js in range(NJS):
                    emit_js(js, accB, 4)
                emit_epilogue(accB, 4, nc.sync, nc.scalar)

    _legalize_multiwait(nc, max_keep=1)
    return nc


_CACHED = {}


def _prep_inputs(x, adj, W, a):
    # x and 16*W packed to fp8 in DoubleRow form [p, ic, .] (d = ic*128+p);
    # the x16 keeps W's small entries out of the fp8 subnormal range and is
    # undone by the Exp input scale / folded into the w column scale.
    xP8 = np.ascontiguousarray(
        x.T.reshape(2, 128, N).transpose(1, 0, 2)
    ).astype(ml_dtypes.float8_e4m3)
    WTe = np.zeros((D, W_FREE), dtype=np.float32)
    WTe[:, :256] = W.T * 16.0
    WTe[:, 256] = (W.T.astype(np.float64) @ a[256:].astype(np.float64)).astype(
        np.float32
    ) * 16.0
    WTeP = np.ascontiguousarray(WTe.reshape(2, 128, W_FREE).transpose(1, 0, 2))
    WTe8 = WTeP.astype(ml_dtypes.float8_e4m3)
    WTe8r = (WTeP - WTe8.astype(np.float32)).astype(ml_dtypes.float8_e4m3)
    # adjacency -> fp8e4 bit patterns (0x00 / 0x38 == 1.0), pre-arranged
    # per core into the [jp, p, b, i, r] device tile layout (j = jp*512 +
    # b*256 + i*128 + p) so every adj DMA is a contiguous copy.
    adj8 = np.where(adj != 0, np.uint8(0x38), np.uint8(0))
    in_maps = []
    for c in range(NCORES):
        adjT_c = np.ascontiguousarray(adj8[c * RB : (c + 1) * RB, :].T)  # [N, RB]
        adjP = adjT_c.reshape(NJP, 2, 2, 128, RB).transpose(0, 3, 1, 2, 4)
        adjP = np.ascontiguousarray(adjP).view(ml_dtypes.float8_e4m3)
        in_maps.append(
            {"xP8": xP8, "WTe8": WTe8, "WTe8r": WTe8r, "adjP8": adjP}
        )
    return in_maps


def _run(in_maps, **kw):
    if "nc" not in _CACHED:
        _CACHED["nc"] = _build_program()
    # The device occasionally comes up wedged (NRT_EXEC_UNIT_UNRECOVERABLE)
    # from a previous process; one retry after a short pause recovers it.
    import time as _time

    last_err = None
    for attempt in range(3):
        try:
            return run_bass_kernel_spmd(
                _CACHED["nc"], in_maps, core_ids=list(range(NCORES)), **kw
            )
        except Exception as e:  # noqa: BLE001
            last_err = e
            if "UNRECOVERABLE" not in str(e) and "UNAVAILABLE" not in str(e):
                raise
            _time.sleep(3.0)
    raise last_err


def _assemble(results):
    blocks = [np.asarray(r["outO"], dtype=np.float32) for r in results]
    return np.concatenate(blocks, axis=0)


def kernel(x, adj, W, a):
    in_maps = _prep_inputs(x, adj, W, a)
    res = _run(in_maps)
    return _assemble(res.results)


# revision 19
# speedup vs baseline: 1.1843x; 1.0148x over previous
"""GAT layer kernel for Trainium2, sharded across 8 NeuronCores.

Math: reference computes
    h = x @ W.T;  e_ij = (h @ a1)[i] + (h @ a2)[j];  mask by adj;
    softmax over j; out = attn @ h.
Because e_i is constant along the softmax axis it cancels, so with
w_j = exp(h_j . a2):
    out[i] = sum_j adj[i,j] * w_j * h[j] / sum_j adj[i,j] * w_j
a1 is mathematically irrelevant.

Design (v15 = v14 + fp8 phase-1):
  * adjacency is packed host-side to fp8e4 BIT PATTERNS (0x00 / 0x38 =
    1.0), pre-arranged per 512-row super-tile into the exact [p, b, i,
    r] device layout so every adj DMA is a pure contiguous copy.
  * THREE bulk DMA streams on separate engine queues, because half A
    consumes adj at ~240 GB/s + x at ~120 GB/s, more than one queue
    delivers: x on the gpsimd SWDGE ring (512KB pieces, first chunk
    split finer for priming), even adj super-tiles on the (otherwise
    idle) sync HWDGE ring emitted upfront, odd adj super-tiles on the
    scalar HWDGE ring with emissions interleaved into the loop so
    they never delay the scalar chain ops.
  * phase 1: h quarters (2 j-chunks) via bf16 matmuls into a PSUM
    scratch rotating over two 2-bank buffers; chain = batched Exp
    (scalar), w-column fp8 cast (scalar), single DVE broadcast
    multiply -> fp8 [w*h | w] tile. Phase-2 consumption lags THREE
    super-chunks so the chain never stalls the PE.
    ws = exp(e + ln(1/8)); the 1/8 keeps w*h inside fp8e4 range and
    cancels in num/den.
  * phase 2 TRANSPOSED vs v10: stationary = adj^T chunk [j:256,
    r:128] (fp8 DoubleRow k=256), moving = [w*h | w] [j:256, 257
    cols]. out[r, 0:256] = numerator rows, out[r, 256] = the
    denominator - it rides along as moving column 257, so v10's
    separate denominator matmuls (a third of the phase-2 moving
    cycles) vanish.
  * PSUM: 4 rotating scratch banks + 4 rotating accumulator banks.
    Rows go in two halves of 4 row-chunks: half A (rows 0:512) fused
    with phase 1; half B (rows 512:1024) a pure phase-2 sweep over
    the SBUF-resident adj tiles, reusing half A's banks right after
    the epilogue-A reads.
  * epilogue per half: DVE reciprocal of the [128, 1] denominator
    column (num and den share the partition = output row), then
    per-partition scale multiplies into one [128, 4, 256] fp16 tile,
    shipped by a single DMA into the final [1024, 256] layout. No
    host math beyond a concat/cast.

Measured numerics: rel err ~9.7e-3 vs fp32 reference (tolerance 2e-2).
"""

import sys

import numpy as np

for _p in ("/opt/trn_rl_repo",):
    try:
        import concourse.bass  # noqa: F401

        break
    except ImportError:
        if _p not in sys.path:
            sys.path.insert(0, _p)

import ml_dtypes

import concourse.bass as bass
import concourse.mybir as mybir
import concourse.tile as tile
from concourse.bass_utils import run_bass_kernel_spmd

dt = mybir.dt
AF = mybir.ActivationFunctionType
PM = mybir.MatmulPerfMode

N = 8192
D = 256
NCORES = 8
RB = N // NCORES  # 1024 output rows per core
W_FREE = 260  # 256 h cols + 1 e col + 3 pad
NJ = N // 128  # 64 j-chunks
NJS = N // 256  # 32 j-super-chunks (DoubleRow k=256)
NJP = NJS // 2  # 16 adj DMA transfers (2 super-chunks each)
HW_FREE = 260  # 256 w*h cols + 1 w col + 3 pad
LAG = 5  # phase-2 consumption lag (in super-chunks) behind phase 1
LOG_S = float(np.log(1.0 / 128.0))  # w scale: num/den both end up /8
ESC = 1.0 / 16.0  # Exp input scale: undoes the x16 on W (fp8 normal range)

# ---------------------------------------------------------------------------
# walrus in this container accepts at most ONE sync-wait command on several
# instruction structs (Drain, 4-byte self-loading Matmult, ...) while the
# newer Tile scheduler emits more. Split the extras into single-wait
# EventSemaphore prefixes on the same engine (identical semantics).
_ev_counter = [0]


def _legalize_multiwait(nc, max_keep=1):
    for f in nc.m.functions:
        for bb in f.blocks:
            il = bb.instructions
            idx = 0
            while idx < len(il):
                inst = il[idx]
                si = inst.sync_info
                if si is not None and si.on_wait and len(si.on_wait) > max_keep:
                    waits = list(si.on_wait)
                    keep = waits[len(waits) - max_keep :] if max_keep else []
                    extra = waits[: len(waits) - max_keep] if max_keep else waits
                    si.on_wait = keep
                    for w in extra:
                        _ev_counter[0] += 1
                        ev = mybir.InstEventSemaphore(
                            name=f"lgw_{_ev_counter[0]}", ins=[], outs=[]
                        )
                        ev.engine = inst.engine
                        ev.sync_info = mybir.SyncInfo(on_wait=[w], on_update=[])
                        il.insert(idx, ev)
                        idx += 1
                idx += 1


# ---------------------------------------------------------------------------


def _build_program():
    nc = bass.Bass("TRN2", debug=False)

    # x packed host-side to fp8 in DoubleRow form: xP8[p, ic, j] = x[j, ic*128+p]
    xP8 = nc.dram_tensor("xP8", [128, 2, N], dt.float8e4, kind="ExternalInput").ap()
    # [16*W.T | 16*W.T@a2] in the same [p, ic, col] DoubleRow form, plus
    # the fp8 quantization RESIDUAL of that matrix: a second accumulating
    # matmul with it cuts the coherent (non-averaging) W quantization error
    # from ~3.5% to ~0.1%.
    WTe8 = nc.dram_tensor(
        "WTe8", [128, 2, W_FREE], dt.float8e4, kind="ExternalInput"
    ).ap()
    WTe8r = nc.dram_tensor(
        "WTe8r", [128, 2, W_FREE], dt.float8e4, kind="ExternalInput"
    ).ap()
    # adj rows of this core, packed host-side into the device tile layout:
    # adjP8[jp, p, b, i, r] = adj^T fp8 pattern for j = jp*512 + b*256 +
    # i*128 + p; each [p, b, i, r] tile is one contiguous 512KB DMA.
    adjP8 = nc.dram_tensor(
        "adjP8", [NJP, 128, 2, 2, RB], dt.float8e4, kind="ExternalInput"
    ).ap()
    # final output rows of this core (fp16 is plenty: |out| <= max|h|,
    # 5e-4 rel step vs 1e-2 budget)
    outO = nc.dram_tensor("outO", [RB, D], dt.float16, kind="ExternalOutput").ap()

    XCH = 1024  # x streamed in [128, 2, XCH] fp8 chunks (one DMA each)
    NXB = N // XCH  # 4 chunks per i-half
    NCPB = XCH // 128  # 16 j-chunks per x chunk

    with tile.TileContext(nc) as tc:
        with (
            tc.tile_pool(name="xr", bufs=1) as xr_pool,
            tc.tile_pool(name="wte", bufs=1) as wte_pool,
            tc.tile_pool(name="hw8", bufs=1) as hw8_pool,
            tc.tile_pool(name="wcol", bufs=4) as w_pool,
            tc.tile_pool(name="adjr", bufs=16) as adj_pool,
            tc.tile_pool(name="eps", bufs=4) as ep_pool,
        ):
            wte8 = wte_pool.tile([128, 2, W_FREE], dt.float8e4, name="wte8")
            nc.scalar.dma_start(wte8, WTe8)
            wte8r = wte_pool.tile([128, 2, W_FREE], dt.float8e4, name="wte8r")
            nc.scalar.dma_start(wte8r, WTe8r)

            xr = [None] * NXB
            at_tiles = [None] * NJP

            # one whole x chunk per DMA: readers wait on every writer of a
            # tile (tile-granular tracking), so finer chunks - not split
            # DMAs - are what actually shortens the critical path.
            def load_x(b, eng):
                eng.dma_start(xr[b], xP8[:, :, b * XCH : (b + 1) * XCH])

            def load_at(jp, engs):
                # one engine: single 512KB DMA; two engines: the two halves
                # land in parallel on both queues (consumers wait for the
                # whole tile either way).
                at = adj_pool.tile(
                    [128, 2, 2, RB], dt.float8e4, name=f"at{jp}", tag="at"
                )
                src = adjP8[jp].rearrange("p b i r -> p (b i r)")
                dst = at.rearrange("p b i r -> p (b i r)")
                step = (2 * 2 * RB) // len(engs)
                for s, eng in enumerate(engs):
                    eng.dma_start(
                        dst[:, s * step : (s + 1) * step],
                        src[:, s * step : (s + 1) * step],
                    )
                at_tiles[jp] = at

            for b in range(NXB):
                xr[b] = xr_pool.tile(
                    [128, 2, XCH], dt.float8e4, name=f"xr{b}", tag="x", bufs=4
                )

            # Bulk DMA split across two rings, each in earliest-deadline
            # order. The serialized sync HWDGE ring (~205 GB/s effective,
            # otherwise idle) carries every other adj super-tile upfront;
            # the gpsimd SWDGE ring (~270 GB/s) interleaves the x pieces
            # with the remaining adj tiles.
            load_x(0, nc.sync)
            load_at(0, (nc.sync, nc.gpsimd))
            load_x(1, nc.gpsimd)
            load_at(1, (nc.sync,))
            load_at(2, (nc.gpsimd,))
            load_x(2, nc.gpsimd)
            load_at(3, (nc.sync,))
            load_x(3, nc.gpsimd)
            load_x(4, nc.sync)
            load_at(4, (nc.gpsimd,))
            load_at(5, (nc.sync,))
            load_x(5, nc.gpsimd)
            load_at(6, (nc.gpsimd,))
            load_at(7, (nc.sync,))
            load_x(6, nc.gpsimd)
            load_at(8, (nc.gpsimd,))
            load_at(9, (nc.sync,))
            load_x(7, nc.gpsimd)
            load_at(10, (nc.gpsimd,))
            load_at(11, (nc.sync,))
            load_at(12, (nc.gpsimd,))
            load_at(13, (nc.sync,))
            load_at(14, (nc.sync,))
            load_at(15, (nc.sync,))

            # fp8 moving tile for phase 2:
            # hw8_all[:, jc, d] = (w*h/8)[j = jc*128 + p, d]  for d < 256
            # hw8_all[:, jc, 256] = (w/8)[j]   (the denominator column)
            hw8_all = hw8_pool.tile([128, NJ, HW_FREE], dt.float8e4, name="hw8_all")
            w_all = w_pool.tile([128, NJ], dt.float32, name="w_all")
            bias_s = w_pool.tile([128, 1], dt.float32, name="bias_s")
            nc.vector.memset(bias_s, LOG_S)

            # ---- fused loop: PSUM = 2x2 rotating scratch banks + 4
            # rotating accumulator banks = exactly 8.
            with (
                tc.tile_pool(name="ph", bufs=1, space="PSUM") as ph_pool,
                tc.tile_pool(name="acc", bufs=1, space="PSUM") as acc_pool,
            ):
                ph_bufs = [
                    ph_pool.tile([128, 2, 512], dt.float32, name=f"ph_q{i}",
                                 tag="ph", bufs=2)
                    for i in range(2)
                ]
                # HAM warm-up: dependency-free matmuls on uninitialized SBUF
                # while the first x/adj DMAs are in flight, so the PE clock
                # gate is already at 8/8 when real work starts. Garbage
                # results land in ph buf 0 and are overwritten by the
                # first real start=True matmul.
                warm = hw8_pool.tile([128, 640], dt.bfloat16, name="warm")
                nc.vector.memset(warm, 0.0)
                for _ in range(3):
                    nc.tensor.matmul(
                        ph_bufs[0][:, 0, :],
                        warm[:, 0:128],
                        warm[:, 128:640],
                        start=True,
                        stop=True,
                    )

                def emit_quarter(q):
                    # h/e matmuls for j-chunks 2q, 2q+1 into the rotating
                    # scratch, then ws = exp(e)/8 and the fp8 moving tile.
                    ph_q = ph_bufs[q % 2]
                    for k in range(2):
                        jc = 2 * q + k
                        b, sl = jc // NCPB, bass.ts(jc % NCPB, 128)
                        for wt, st in ((wte8, True), (wte8r, False)):
                            nc.tensor.matmul(
                                ph_q[:, k, 0:W_FREE],
                                xr[b][:, :, sl],
                                wt,
                                start=st,
                                stop=not st,
                                perf_mode=PM.DoubleRow,
                            )
                    j0 = 2 * q
                    nc.scalar.activation(
                        w_all[:, j0 : j0 + 2],
                        ph_q[:, :, 256],
                        AF.Exp,
                        bias=bias_s[:, 0:1],
                        scale=ESC,
                    )
                    # den column = 16*w_all = w/8  (ph holds 16h, w_all = w/128,
                    # so hw8 = ph*w_all = w*h/8 and num/den matches exactly)
                    nc.scalar.activation(
                        hw8_all[:, j0 : j0 + 2, 256],
                        w_all[:, j0 : j0 + 2],
                        AF.Copy,
                        scale=16.0,
                    )
                    nc.vector.tensor_tensor(
                        hw8_all[:, j0 : j0 + 2, 0:256],
                        ph_q[:, :, 0:256],
                        w_all[:, j0 : j0 + 2].to_broadcast([128, 2, 256]),
                        mybir.AluOpType.mult,
                    )

                def emit_js(js, accs, rc0):
                    # phase-2 matmuls for super-chunk js, row-chunks
                    # rc0..rc0+3: stationary = adj^T [256, 128], moving =
                    # [w*h | w] [256, 257]; out[r, 0:256]=num, out[r,256]=den
                    at = at_tiles[js // 2][:, js % 2]  # [128, 2, RB]
                    st, sp = js == 0, js == NJS - 1
                    mov = hw8_all[:, 2 * js : 2 * js + 2, 0:257]
                    for k, acc in enumerate(accs):
                        rc = rc0 + k
                        nc.tensor.matmul(
                            acc[:, 0:257],
                            at[:, :, rc * 128 : (rc + 1) * 128],
                            mov,
                            start=st,
                            stop=sp,
                            perf_mode=PM.DoubleRow,
                            skip_group_check=True,
                        )

                def emit_epilogue(accs, rc0, eng, eng2=None):
                    # num and den share the partition (= output row): DVE
                    # reciprocal of the [128,1] den column, then per-
                    # partition scale multiplies -> fp16; each row-chunk
                    # ships as soon as its multiply lands, alternating DMA
                    # queues so the transfers overlap the remaining math.
                    ob = ep_pool.tile([128, 4, 256], dt.float16, name="ob", tag="ob")
                    for k, acc in enumerate(accs):
                        rec = ep_pool.tile([128, 1], dt.float32, name="rec", tag="rec")
                        nc.vector.reciprocal(rec, acc[:, 256:257])
                        if k % 2 == 0:
                            nc.scalar.activation(
                                ob[:, k, :], acc[:, 0:256], AF.Copy, scale=rec[:, 0:1]
                            )
                        else:
                            nc.vector.tensor_scalar_mul(
                                ob[:, k, :], acc[:, 0:256], rec[:, 0:1]
                            )
                        rc = rc0 + k
                        dst = outO[rc * 128 : (rc + 1) * 128, :]
                        demg = eng if (eng2 is None or k % 2 == 0) else eng2
                        demg.dma_start(dst, ob[:, k, :])

                # ---- half A: rows 0:512 fused with phase 1; phase-2
                # consumption lags LAG super-chunks so the convert chain
                # never stalls the PE. Odd adj tiles are emitted on the
                # scalar ring two tiles ahead of consumption.
                accA = [
                    acc_pool.tile([128, 512], dt.float32, name=f"accA{rc}",
                                  tag="acc", bufs=4)
                    for rc in range(4)
                ]
                for q in range(NJS + LAG):
                    if q < NJS:
                        if q >= LAG:
                            emit_js(q - LAG, accA, 0)
                        emit_quarter(q)
                    else:
                        emit_js(q - LAG, accA, 0)
                emit_epilogue(accA, 0, nc.sync)

                # ---- half B: rows 512:1024, pure phase-2 sweep; the
                # accumulators rotate onto half A's banks, whose epilogue
                # reads are already done (they were emitted first).
                accB = [
                    acc_pool.tile([128, 512], dt.float32, name=f"accB{rc}",
                                  tag="acc", bufs=4)
                    for rc in range(4)
                ]
                for js in range(NJS):
                    emit_js(js, accB, 4)
                emit_epilogue(accB, 4, nc.sync, nc.scalar)

    _legalize_multiwait(nc, max_keep=1)
    return nc


_CACHED = {}


def _prep_inputs(x, adj, W, a):
    # x and 16*W packed to fp8 in DoubleRow form [p, ic, .] (d = ic*128+p);
    # the x16 keeps W's small entries out of the fp8 subnormal range and is
    # undone by the Exp input scale / folded into the w column scale.
    xP8 = np.ascontiguousarray(
        x.T.reshape(2, 128, N).transpose(1, 0, 2)
    ).astype(ml_dtypes.float8_e4m3)
    WTe = np.zeros((D, W_FREE), dtype=np.float32)
    WTe[:, :256] = W.T * 16.0
    WTe[:, 256] = (W.T.astype(np.float64) @ a[256:].astype(np.float64)).astype(
        np.float32
    ) * 16.0
    WTeP = np.ascontiguousarray(WTe.reshape(2, 128, W_FREE).transpose(1, 0, 2))
    WTe8 = WTeP.astype(ml_dtypes.float8_e4m3)
    WTe8r = (WTeP - WTe8.astype(np.float32)).astype(ml_dtypes.float8_e4m3)
    # adjacency -> fp8e4 bit patterns (0x00 / 0x38 == 1.0), pre-arranged
    # per core into the [jp, p, b, i, r] device tile layout (j = jp*512 +
    # b*256 + i*128 + p) so every adj DMA is a contiguous copy.
    adj8 = np.where(adj != 0, np.uint8(0x38), np.uint8(0))
    in_maps = []
    for c in range(NCORES):
        adjT_c = np.ascontiguousarray(adj8[c * RB : (c + 1) * RB, :].T)  # [N, RB]
        adjP = adjT_c.reshape(NJP, 2, 2, 128, RB).transpose(0, 3, 1, 2, 4)
        adjP = np.ascontiguousarray(adjP).view(ml_dtypes.float8_e4m3)
        in_maps.append(
            {"xP8": xP8, "WTe8": WTe8, "WTe8r": WTe8r, "adjP8": adjP}
        )
    return in_maps


def _run(in_maps, **kw):
    if "nc" not in _CACHED:
        _CACHED["nc"] = _build_program()
    # The device occasionally comes up wedged (NRT_EXEC_UNIT_UNRECOVERABLE)
    # from a previous process; one retry after a short pause recovers it.
    import time as _time

    last_err = None
    for attempt in range(3):
        try:
            return run_bass_kernel_spmd(
                _CACHED["nc"], in_maps, core_ids=list(range(NCORES)), **kw
            )
        except Exception as e:  # noqa: BLE001
            last_err = e
            if "UNRECOVERABLE" not in str(e) and "UNAVAILABLE" not in str(e):
                raise
            _time.sleep(3.0)
    raise last_err


def _assemble(results):
    blocks = [np.asarray(r["outO"], dtype=np.float32) for r in results]
    return np.concatenate(blocks, axis=0)


def kernel(x, adj, W, a):
    in_maps = _prep_inputs(x, adj, W, a)
    res = _run(in_maps)
    return _assemble(res.results)


# revision 20
# speedup vs baseline: 1.1928x; 1.0072x over previous
"""GAT layer kernel for Trainium2, sharded across 8 NeuronCores.

Math: reference computes
    h = x @ W.T;  e_ij = (h @ a1)[i] + (h @ a2)[j];  mask by adj;
    softmax over j; out = attn @ h.
Because e_i is constant along the softmax axis it cancels, so with
w_j = exp(h_j . a2):
    out[i] = sum_j adj[i,j] * w_j * h[j] / sum_j adj[i,j] * w_j
a1 is mathematically irrelevant.

Design (v23):
  * adjacency is packed host-side to fp8e4 BIT PATTERNS (0x00 / 0x38 =
    1.0), pre-arranged per 512-row super-tile into the exact [p, b, i,
    r] device layout so every adj DMA is a pure contiguous copy.
  * phase 1 in fp8: x and 16*W (the x16 keeps W's entries out of the
    fp8 subnormal range) are packed host-side into DoubleRow [p, ic, .]
    form, so h arrives via ONE k=256 fp8 matmul per 128-j chunk - plus
    a second accumulating matmul with the fp8 quantization RESIDUAL of
    16*W, which cuts the coherent (non-averaging) W error from ~3.5%
    to ~0.1%. The e = h.a2 column rides along as moving column 256.
  * chain per quarter (2 j-chunks): batched Exp on scalar ACT (input
    scale 1/16 undoes the x16; bias ln(1/128)), fp8 w-column cast with
    scale 16 (so den = w/8), one DVE broadcast multiply -> the fp8
    [w*h | w] moving tile (hw8 = 16h * w/128 = w*h/8). Phase-2
    consumption lags FIVE super-chunks so the chain and the adj
    stream never stall the PE; the PSUM scratch rotates over two
    2-bank buffers.
  * phase 2: stationary = adj^T chunk [j:256, r:128] (fp8 DoubleRow
    k=256), moving = [w*h | w] [j:256, 257 cols]. out[r, 0:256] =
    numerator rows, out[r, 256] = the denominator - it rides along as
    moving column 257, so there are NO separate denominator passes.
  * PSUM: 4 rotating scratch banks + 4 rotating accumulator banks.
    Rows go in two halves of 4 row-chunks: half A (rows 0:512) fused
    with phase 1; half B (rows 512:1024) a pure phase-2 sweep over the
    SBUF-resident adj tiles, reusing half A's banks right after the
    epilogue-A reads.
  * bulk DMA on TWO engine queues (per-core HBM supply is ~300-360
    GB/s shared with the other 7 cores; one queue sustains only
    ~160-250), in earliest-deadline order: the sync HWDGE ring carries
    x0 + every other adj super-tile upfront; the gpsimd SWDGE ring
    interleaves the remaining x chunks (256KB each - readers wait on
    every writer of a tile, so fine CHUNKS, not split DMAs, shorten
    the critical path) with the other adj tiles. at0's halves land in
    parallel on both queues.
  * epilogue per half: DVE reciprocal of the [128, 1] denominator
    column (num and den share the partition = output row), per-
    partition scale multiplies -> fp16, each row-chunk DMA'd as soon
    as its multiply lands, alternating queues. No host math beyond a
    concat/cast.

Measured numerics: rel err ~1.46e-2 vs fp32 reference (tolerance 2e-2).
Measured HW exec: ~70.2-71.2us (baseline v10: 82.5us).
"""

import sys

import numpy as np

for _p in ("/opt/trn_rl_repo",):
    try:
        import concourse.bass  # noqa: F401

        break
    except ImportError:
        if _p not in sys.path:
            sys.path.insert(0, _p)

import ml_dtypes

import concourse.bass as bass
import concourse.mybir as mybir
import concourse.tile as tile
from concourse.bass_utils import run_bass_kernel_spmd

dt = mybir.dt
AF = mybir.ActivationFunctionType
PM = mybir.MatmulPerfMode

N = 8192
D = 256
NCORES = 8
RB = N // NCORES  # 1024 output rows per core
W_FREE = 260  # 256 h cols + 1 e col + 3 pad
NJ = N // 128  # 64 j-chunks
NJS = N // 256  # 32 j-super-chunks (DoubleRow k=256)
NJP = NJS // 2  # 16 adj DMA transfers (2 super-chunks each)
HW_FREE = 260  # 256 w*h cols + 1 w col + 3 pad
LAG = 5  # phase-2 consumption lag (in super-chunks) behind phase 1
LOG_S = float(np.log(1.0 / 128.0))  # w scale: num/den both end up /8
ESC = 1.0 / 16.0  # Exp input scale: undoes the x16 on W (fp8 normal range)

# ---------------------------------------------------------------------------
# walrus in this container accepts at most ONE sync-wait command on several
# instruction structs (Drain, 4-byte self-loading Matmult, ...) while the
# newer Tile scheduler emits more. Split the extras into single-wait
# EventSemaphore prefixes on the same engine (identical semantics).
_ev_counter = [0]


def _legalize_multiwait(nc, max_keep=1):
    for f in nc.m.functions:
        for bb in f.blocks:
            il = bb.instructions
            idx = 0
            while idx < len(il):
                inst = il[idx]
                si = inst.sync_info
                if si is not None and si.on_wait and len(si.on_wait) > max_keep:
                    waits = list(si.on_wait)
                    keep = waits[len(waits) - max_keep :] if max_keep else []
                    extra = waits[: len(waits) - max_keep] if max_keep else waits
                    si.on_wait = keep
                    for w in extra:
                        _ev_counter[0] += 1
                        ev = mybir.InstEventSemaphore(
                            name=f"lgw_{_ev_counter[0]}", ins=[], outs=[]
                        )
                        ev.engine = inst.engine
                        ev.sync_info = mybir.SyncInfo(on_wait=[w], on_update=[])
                        il.insert(idx, ev)
                        idx += 1
                idx += 1


# ---------------------------------------------------------------------------


def _build_program():
    nc = bass.Bass("TRN2", debug=False)

    # x packed host-side to fp8 in DoubleRow form: xP8[p, ic, j] = x[j, ic*128+p]
    xP8 = nc.dram_tensor("xP8", [128, 2, N], dt.float8e4, kind="ExternalInput").ap()
    # [16*W.T | 16*W.T@a2] in the same [p, ic, col] DoubleRow form, plus
    # the fp8 quantization RESIDUAL of that matrix: a second accumulating
    # matmul with it cuts the coherent (non-averaging) W quantization error
    # from ~3.5% to ~0.1%.
    WTe8 = nc.dram_tensor(
        "WTe8", [128, 2, W_FREE], dt.float8e4, kind="ExternalInput"
    ).ap()
    WTe8r = nc.dram_tensor(
        "WTe8r", [128, 2, W_FREE], dt.float8e4, kind="ExternalInput"
    ).ap()
    # adj rows of this core, packed host-side into the device tile layout:
    # adjP8[jp, p, b, i, r] = adj^T fp8 pattern for j = jp*512 + b*256 +
    # i*128 + p; each [p, b, i, r] tile is one contiguous 512KB DMA.
    adjP8 = nc.dram_tensor(
        "adjP8", [NJP, 128, 2, 2, RB], dt.float8e4, kind="ExternalInput"
    ).ap()
    # final output rows of this core (fp16 is plenty: |out| <= max|h|,
    # 5e-4 rel step vs 1e-2 budget)
    outO = nc.dram_tensor("outO", [RB, D], dt.float16, kind="ExternalOutput").ap()

    XCH = 1024  # x streamed in [128, 2, XCH] fp8 chunks (one DMA each)
    NXB = N // XCH  # 4 chunks per i-half
    NCPB = XCH // 128  # 16 j-chunks per x chunk

    with tile.TileContext(nc) as tc:
        with (
            tc.tile_pool(name="xr", bufs=1) as xr_pool,
            tc.tile_pool(name="wte", bufs=1) as wte_pool,
            tc.tile_pool(name="hw8", bufs=1) as hw8_pool,
            tc.tile_pool(name="wcol", bufs=4) as w_pool,
            tc.tile_pool(name="adjr", bufs=16) as adj_pool,
            tc.tile_pool(name="eps", bufs=4) as ep_pool,
        ):
            wte8 = wte_pool.tile([128, 2, W_FREE], dt.float8e4, name="wte8")
            nc.scalar.dma_start(wte8, WTe8)
            wte8r = wte_pool.tile([128, 2, W_FREE], dt.float8e4, name="wte8r")
            nc.scalar.dma_start(wte8r, WTe8r)

            xr = [None] * NXB
            at_tiles = [None] * NJP

            # one whole x chunk per DMA: readers wait on every writer of a
            # tile (tile-granular tracking), so finer chunks - not split
            # DMAs - are what actually shortens the critical path.
            def load_x(b, eng):
                eng.dma_start(xr[b], xP8[:, :, b * XCH : (b + 1) * XCH])

            def load_at(jp, engs):
                # one engine: single 512KB DMA; two engines: the two halves
                # land in parallel on both queues (consumers wait for the
                # whole tile either way).
                at = adj_pool.tile(
                    [128, 2, 2, RB], dt.float8e4, name=f"at{jp}", tag="at"
                )
                src = adjP8[jp].rearrange("p b i r -> p (b i r)")
                dst = at.rearrange("p b i r -> p (b i r)")
                step = (2 * 2 * RB) // len(engs)
                for s, eng in enumerate(engs):
                    eng.dma_start(
                        dst[:, s * step : (s + 1) * step],
                        src[:, s * step : (s + 1) * step],
                    )
                at_tiles[jp] = at

            for b in range(NXB):
                xr[b] = xr_pool.tile(
                    [128, 2, XCH], dt.float8e4, name=f"xr{b}", tag="x", bufs=4
                )

            # Bulk DMA split across two rings, each in earliest-deadline
            # order. The serialized sync HWDGE ring (~205 GB/s effective,
            # otherwise idle) carries every other adj super-tile upfront;
            # the gpsimd SWDGE ring (~270 GB/s) interleaves the x pieces
            # with the remaining adj tiles.
            load_x(0, nc.sync)
            load_at(0, (nc.sync, nc.gpsimd))
            load_x(1, nc.gpsimd)
            load_at(1, (nc.sync,))
            load_at(2, (nc.gpsimd,))
            load_x(2, nc.gpsimd)
            load_at(3, (nc.sync,))
            load_x(3, nc.gpsimd)
            load_x(4, nc.sync)
            load_at(4, (nc.gpsimd,))
            load_at(5, (nc.sync,))
            load_x(5, nc.gpsimd)
            load_at(6, (nc.gpsimd,))
            load_at(7, (nc.sync,))
            load_x(6, nc.gpsimd)
            load_at(8, (nc.gpsimd,))
            load_at(9, (nc.sync,))
            load_x(7, nc.gpsimd)
            load_at(10, (nc.gpsimd,))
            load_at(11, (nc.sync,))
            load_at(12, (nc.gpsimd,))
            load_at(13, (nc.sync,))
            load_at(14, (nc.sync,))
            load_at(15, (nc.sync,))

            # fp8 moving tile for phase 2:
            # hw8_all[:, jc, d] = (w*h/8)[j = jc*128 + p, d]  for d < 256
            # hw8_all[:, jc, 256] = (w/8)[j]   (the denominator column)
            hw8_all = hw8_pool.tile([128, NJ, HW_FREE], dt.float8e4, name="hw8_all")
            w_all = w_pool.tile([128, NJ], dt.float32, name="w_all")
            bias_s = w_pool.tile([128, 1], dt.float32, name="bias_s")
            nc.vector.memset(bias_s, LOG_S)

            # ---- fused loop: PSUM = 2x2 rotating scratch banks + 4
            # rotating accumulator banks = exactly 8.
            with (
                tc.tile_pool(name="ph", bufs=1, space="PSUM") as ph_pool,
                tc.tile_pool(name="acc", bufs=1, space="PSUM") as acc_pool,
            ):
                ph_bufs = [
                    ph_pool.tile([128, 2, 512], dt.float32, name=f"ph_q{i}",
                                 tag="ph", bufs=2)
                    for i in range(2)
                ]
                # HAM warm-up: dependency-free matmuls on uninitialized SBUF
                # while the first x/adj DMAs are in flight, so the PE clock
                # gate is already at 8/8 when real work starts. Garbage
                # results land in ph buf 0 and are overwritten by the
                # first real start=True matmul.
                warm = hw8_pool.tile([128, 640], dt.bfloat16, name="warm")
                nc.vector.memset(warm, 0.0)
                for _ in range(3):
                    nc.tensor.matmul(
                        ph_bufs[0][:, 0, :],
                        warm[:, 0:128],
                        warm[:, 128:640],
                        start=True,
                        stop=True,
                    )

                def emit_quarter(q):
                    # h/e matmuls for j-chunks 2q, 2q+1 into the rotating
                    # scratch, then ws = exp(e)/8 and the fp8 moving tile.
                    ph_q = ph_bufs[q % 2]
                    for k in range(2):
                        jc = 2 * q + k
                        b, sl = jc // NCPB, bass.ts(jc % NCPB, 128)
                        for wt, st in ((wte8, True), (wte8r, False)):
                            nc.tensor.matmul(
                                ph_q[:, k, 0:W_FREE],
                                xr[b][:, :, sl],
                                wt,
                                start=st,
                                stop=not st,
                                perf_mode=PM.DoubleRow,
                            )
                    j0 = 2 * q
                    nc.scalar.activation(
                        w_all[:, j0 : j0 + 2],
                        ph_q[:, :, 256],
                        AF.Exp,
                        bias=bias_s[:, 0:1],
                        scale=ESC,
                    )
                    # den column = 16*w_all = w/8  (ph holds 16h, w_all = w/128,
                    # so hw8 = ph*w_all = w*h/8 and num/den matches exactly)
                    nc.scalar.activation(
                        hw8_all[:, j0 : j0 + 2, 256],
                        w_all[:, j0 : j0 + 2],
                        AF.Copy,
                        scale=16.0,
                    )
                    nc.vector.tensor_tensor(
                        hw8_all[:, j0 : j0 + 2, 0:256],
                        ph_q[:, :, 0:256],
                        w_all[:, j0 : j0 + 2].to_broadcast([128, 2, 256]),
                        mybir.AluOpType.mult,
                    )

                def emit_js(js, accs, rc0):
                    # phase-2 matmuls for super-chunk js, row-chunks
                    # rc0..rc0+3: stationary = adj^T [256, 128], moving =
                    # [w*h | w] [256, 257]; out[r, 0:256]=num, out[r,256]=den
                    at = at_tiles[js // 2][:, js % 2]  # [128, 2, RB]
                    st, sp = js == 0, js == NJS - 1
                    mov = hw8_all[:, 2 * js : 2 * js + 2, 0:257]
                    for k, acc in enumerate(accs):
                        rc = rc0 + k
                        nc.tensor.matmul(
                            acc[:, 0:257],
                            at[:, :, rc * 128 : (rc + 1) * 128],
                            mov,
                            start=st,
                            stop=sp,
                            perf_mode=PM.DoubleRow,
                            skip_group_check=True,
                        )

                def emit_epilogue(accs, rc0, eng, eng2=None):
                    # num and den share the partition (= output row): DVE
                    # reciprocal of the [128,1] den column, then per-
                    # partition scale multiplies -> fp16; each row-chunk
                    # ships as soon as its multiply lands, alternating DMA
                    # queues so the transfers overlap the remaining math.
                    ob = ep_pool.tile([128, 4, 256], dt.float16, name="ob", tag="ob")
                    for k, acc in enumerate(accs):
                        rec = ep_pool.tile([128, 1], dt.float32, name="rec", tag="rec")
                        nc.vector.reciprocal(rec, acc[:, 256:257])
                        if k % 2 == 0:
                            nc.scalar.activation(
                                ob[:, k, :], acc[:, 0:256], AF.Copy, scale=rec[:, 0:1]
                            )
                        else:
                            nc.vector.tensor_scalar_mul(
                                ob[:, k, :], acc[:, 0:256], rec[:, 0:1]
                            )
                        rc = rc0 + k
                        dst = outO[rc * 128 : (rc + 1) * 128, :]
                        demg = eng if (eng2 is None or k % 2 == 0) else eng2
                        demg.dma_start(dst, ob[:, k, :])

                # ---- half A: rows 0:512 fused with phase 1; phase-2
                # consumption lags LAG super-chunks so the convert chain
                # never stalls the PE. Odd adj tiles are emitted on the
                # scalar ring two tiles ahead of consumption.
                accA = [
                    acc_pool.tile([128, 512], dt.float32, name=f"accA{rc}",
                                  tag="acc", bufs=4)
                    for rc in range(4)
                ]
                for q in range(NJS + LAG):
                    if q < NJS:
                        if q >= LAG:
                            emit_js(q - LAG, accA, 0)
                        emit_quarter(q)
                    else:
                        emit_js(q - LAG, accA, 0)
                emit_epilogue(accA, 0, nc.sync)

                # ---- half B: rows 512:1024, pure phase-2 sweep; the
                # accumulators rotate onto half A's banks, whose epilogue
                # reads are already done (they were emitted first).
                accB = [
                    acc_pool.tile([128, 512], dt.float32, name=f"accB{rc}",
                                  tag="acc", bufs=4)
                    for rc in range(4)
                ]
                for js in range(NJS):
                    emit_js(js, accB, 4)
                emit_epilogue(accB, 4, nc.sync, nc.scalar)

    _legalize_multiwait(nc, max_keep=1)
    return nc


_CACHED = {}


def _prep_inputs(x, adj, W, a):
    # x and 16*W packed to fp8 in DoubleRow form [p, ic, .] (d = ic*128+p);
    # the x16 keeps W's small entries out of the fp8 subnormal range and is
    # undone by the Exp input scale / folded into the w column scale.
    xP8 = np.ascontiguousarray(
        x.T.reshape(2, 128, N).transpose(1, 0, 2)
    ).astype(ml_dtypes.float8_e4m3)
    WTe = np.zeros((D, W_FREE), dtype=np.float32)
    WTe[:, :256] = W.T * 16.0
    WTe[:, 256] = (W.T.astype(np.float64) @ a[256:].astype(np.float64)).astype(
        np.float32
    ) * 16.0
    WTeP = np.ascontiguousarray(WTe.reshape(2, 128, W_FREE).transpose(1, 0, 2))
    WTe8 = WTeP.astype(ml_dtypes.float8_e4m3)
    WTe8r = (WTeP - WTe8.astype(np.float32)).astype(ml_dtypes.float8_e4m3)
    # adjacency -> fp8e4 bit patterns (0x00 / 0x38 == 1.0), pre-arranged
    # per core into the [jp, p, b, i, r] device tile layout (j = jp*512 +
    # b*256 + i*128 + p) so every adj DMA is a contiguous copy.
    adj8 = np.where(adj != 0, np.uint8(0x38), np.uint8(0))
    in_maps = []
    for c in range(NCORES):
        adjT_c = np.ascontiguousarray(adj8[c * RB : (c + 1) * RB, :].T)  # [N, RB]
        adjP = adjT_c.reshape(NJP, 2, 2, 128, RB).transpose(0, 3, 1, 2, 4)
        adjP = np.ascontiguousarray(adjP).view(ml_dtypes.float8_e4m3)
        in_maps.append(
            {"xP8": xP8, "WTe8": WTe8, "WTe8r": WTe8r, "adjP8": adjP}
        )
    return in_maps


def _run(in_maps, **kw):
    if "nc" not in _CACHED:
        _CACHED["nc"] = _build_program()
    # The device occasionally comes up wedged (NRT_EXEC_UNIT_UNRECOVERABLE)
    # from a previous process; one retry after a short pause recovers it.
    import time as _time

    last_err = None
    for attempt in range(3):
        try:
            return run_bass_kernel_spmd(
                _CACHED["nc"], in_maps, core_ids=list(range(NCORES)), **kw
            )
        except Exception as e:  # noqa: BLE001
            last_err = e
            if "UNRECOVERABLE" not in str(e) and "UNAVAILABLE" not in str(e):
                raise
            _time.sleep(3.0)
    raise last_err


def _assemble(results):
    blocks = [np.asarray(r["outO"], dtype=np.float32) for r in results]
    return np.concatenate(blocks, axis=0)


def kernel(x, adj, W, a):
    in_maps = _prep_inputs(x, adj, W, a)
    res = _run(in_maps)
    return _assemble(res.results)
